# revision 33
# baseline (speedup 1.0000x reference)
"""Multi-head attention kernel for Trainium2, 8 NeuronCores.

Problem: B=4, T=2048, D_in=1024, 16 heads x 64 dim, E=1024 (fp32).

Sharding: (batch x head-group). Core c handles batch b=c//2 and head-group
g=c%2 (8 heads, 512 qk/v dims). Each core computes its batch's QKV
projections restricted to its heads, full attention for those heads, and a
partial output projection. The host sums the two partial projections per
batch (the only cross-core reduction) and stacks batches.

Per-core dataflow (all matmuls bf16 inputs, fp32 PSUM accumulation):
  xT      = dma-xbar-transpose(cast_bf16(x))            [1024, 2048] per tensor
  qhT/khT = w.T @ xT   (weights stationary)             [512, 2048]
  vh_ext  = xT.T @ wv + ones column                     [2048, 8*65]
  S^T     = khT_h.T @ qhT_h per head pair               PSUM [128,1024]
  expS    = ACT exp(S^T/8) -> bf16 SBUF                 (the softmax exp)
  AVt     = es_block.T @ vh_ext  ("flipped": es is the stationary operand,
            the [128,65] v-slab is the moving operand)  PSUM [tq=128, 65]
            col 64 = softmax denominator (ones column)
  attn_n  = AVt[:, 0:64] * recip(AVt[:, 64])            per-partition scalar
  attnT   = dma-xbar-transpose(attn_n via DRAM)         d-major, per 512-q chunk
  y      += attnT_m.T @ wp_m  (K=128 contraction)       [2048, 1024] fp32

Two scheduling constraints shape the emission order, which a small
build-time list scheduler produces from per-engine clock models:
 1. The scalar-engine exp stream (~1.04us per [128,1024] tile) must run
    continuously; S matmuls pace it and QKV/projection chains fill the
    remaining PE slack.
 2. The Tile framework serializes DMA completions into a near-total order
    through 8+8 recycled DMA semaphores, costing ~2.4us of dead pipe time
    per DMA on top of the transfer, so the kernel uses few, large DMAs.
"""

import sys

import numpy as np

if "/opt/trn_rl_repo" not in sys.path:
    sys.path.insert(0, "/opt/trn_rl_repo")

B, T, DIN = 4, 2048, 1024
NH, HD, EMB = 16, 64, 1024
HGD = 512          # per-core qk/v dims (8 heads * 64)
NKT = DIN // 128   # 8  input-dim k tiles
NQC = T // 512     # 4  t chunks of 512
NTT = T // 128     # 16 t tiles of 128
NM = HGD // 128    # 4  head-pair m tiles
HPC = 8            # heads per core
NW = 16            # windows (qc, pair)

_CACHE = {}

# clock model (ns): used only to choose emission order; correctness is
# semaphore-driven regardless of these estimates
MM = 1.0 / 2.4            # ns per moving column (bf16, warm PE)
EXP_NS = 1080.0           # ACT exp of a [128, 1024] tile
SEM = 170.0
DMA_LINK = 2400.0         # dead time per DMA from the pinned completion chain
ES_BUFS = 18
XT_BUFS = 6
DVE_COPY = 660.0


def _build_nc():
    import concourse.bacc as bacc
    import concourse.mybir as mybir
    import concourse.tile as tile
    from contextlib import ExitStack

    dt = mybir.dt
    AF = mybir.ActivationFunctionType

    nc = bacc.Bacc("TRN2", target_bir_lowering=False, debug=False)
    xq = nc.declare_dram_parameter("xq", [T, DIN], dt.float32, isOutput=False)
    xk = nc.declare_dram_parameter("xk", [T, DIN], dt.float32, isOutput=False)
    xv = nc.declare_dram_parameter("xv", [T, DIN], dt.float32, isOutput=False)
    wq = nc.declare_dram_parameter("wq", [DIN, HGD], dt.float32, isOutput=False)
    wk = nc.declare_dram_parameter("wk", [DIN, HGD], dt.float32, isOutput=False)
    wv = nc.declare_dram_parameter("wv", [DIN, HGD], dt.float32, isOutput=False)
    wp = nc.declare_dram_parameter("wp", [HGD, EMB], dt.float32, isOutput=False)
    y = nc.declare_dram_parameter("y", [T, EMB], dt.float32, isOutput=True)

    with tile.TileContext(nc) as tc, ExitStack() as ctx:
        p_w = ctx.enter_context(tc.tile_pool(name="weights", bufs=1))
        p_xt = ctx.enter_context(tc.tile_pool(name="xt", bufs=XT_BUFS))
        p_qkh = ctx.enter_context(tc.tile_pool(name="qkh", bufs=1))
        p_vh = ctx.enter_context(tc.tile_pool(name="vh", bufs=1))
        p_exps = ctx.enter_context(tc.tile_pool(name="exps", bufs=ES_BUFS))
        p_attn = ctx.enter_context(tc.tile_pool(name="attn", bufs=1))
        p_an = ctx.enter_context(tc.tile_pool(name="attn_n", bufs=2))
        p_norm = ctx.enter_context(tc.tile_pool(name="norm", bufs=4))
        p_y = ctx.enter_context(tc.tile_pool(name="ysb", bufs=2))
        p_ps = ctx.enter_context(tc.tile_pool(name="psum_s", bufs=2, space="PSUM"))
        p_av = ctx.enter_context(tc.tile_pool(name="psum_av", bufs=1, space="PSUM"))
        p_big = ctx.enter_context(tc.tile_pool(name="psum_big", bufs=1, space="PSUM"))

        # bf16 copies of the inputs (DRAM->DRAM cast), transposed-read later
        xqb = nc.dram_tensor("xqb", [T, DIN], dt.bfloat16)
        xkb = nc.dram_tensor("xkb", [T, DIN], dt.bfloat16)
        xvb = nc.dram_tensor("xvb", [T, DIN], dt.bfloat16)
        # normalized attention, t-major, staged for the xbar transpose
        attn_d = nc.dram_tensor("attn_d", [T, HGD], dt.bfloat16)

        # --- persistent SBUF ---
        wq_sb = p_w.tile([128, NKT, HGD], dt.bfloat16, tag="wq")
        wk_sb = p_w.tile([128, NKT, HGD], dt.bfloat16, tag="wk")
        wv_sb = p_w.tile([128, NKT, HGD], dt.bfloat16, tag="wv")
        wp_sb = p_w.tile([128, NM, EMB], dt.bfloat16, tag="wp")

        qhT = [p_qkh.tile([128, T], dt.bfloat16, tag=f"qhT{m}", name=f"qhT{m}") for m in range(NM)]
        khT = [p_qkh.tile([128, T], dt.bfloat16, tag=f"khT{m}", name=f"khT{m}") for m in range(NM)]
        vh_ext = [p_vh.tile([128, HPC, HD + 1], dt.bfloat16, tag=f"vh{t_}", name=f"vh{t_}") for t_ in range(NTT)]
        for t_ in range(NTT):
            nc.vector.memset(vh_ext[t_][:, :, HD : HD + 1], 1.0)
        # single d-major attention tile: attnT[p, m, t] = attn[t, m*128+p]
        attnT = p_attn.tile([128, NM, T], dt.bfloat16, tag="attnT")
        # flipped-AV accumulators: two ping-pong PSUM banks, each holding
        # exactly ONE [128, 65] accumulator at a time so start=True zeroing
        # semantics are bank-exact on real hardware
        av_banks = (
            p_av.tile([128, HD + 1], dt.float32, tag="ava", name="ava"),
            p_av.tile([128, HD + 1], dt.float32, tag="avb", name="avb"),
            p_av.tile([128, HD + 1], dt.float32, tag="avc", name="avc"),
        )

        # ================= staging: casts (SWDGE) + xposes (SP) ============
        # Few, large DMAs: 6 input casts (block 0 + blocks 1-3 per tensor),
        # 4 direct weight loads, then one XPOSE per 512-token block.
        pipe = [0.0]
        xts = {}    # (tensor, block) -> xt tile
        n_load_T = [0]
        srcs = {"q": (xqb, xq), "k": (xkb, xk), "v": (xvb, xv)}
        cast_est = {}

        def cast(tname, lo, hi):
            # reshape rows x1024 -> rows/4 x 4096 to keep the SWDGE ring happy
            xb_d, xs = srcs[tname]
            nc.gpsimd.dma_start(
                out=xb_d[lo:hi, :].rearrange("(a r) b -> a (r b)", r=4),
                in_=xs[lo:hi, :].rearrange("(a r) b -> a (r b)", r=4),
            )
            pipe[0] += (hi - lo) * DIN * 2 / 360.0 + DMA_LINK
            cast_est[(tname, lo)] = pipe[0]

        def wload(dst, src, pat):
            nc.gpsimd.dma_start(out=dst[:], in_=src.rearrange(pat, p=128))
            pipe[0] += 2912 + DMA_LINK
            return pipe[0] + 1500

        west = {}
        cast("q", 0, 512)
        cast("k", 0, 512)
        west["wq"] = wload(wq_sb, wq, "(kt p) n -> p kt n")
        west["wk"] = wload(wk_sb, wk, "(kt p) n -> p kt n")
        cast("k", 512, 1024)
        cast("v", 0, 512)
        west["wv"] = wload(wv_sb, wv, "(kt p) n -> p kt n")
        cast("v", 512, 1024)
        cast("q", 512, 2048)
        cast("k", 1024, 2048)
        cast("v", 1024, 2048)
        west["wp"] = wload(wp_sb, wp, "(m p) e -> p m e")

        def block_cast_key(tname, b):
            if b == 0:
                return (tname, 0)
            if tname == "q":
                return (tname, 512)
            return (tname, 512 if b == 1 else 1024)

        # XPOSEs are deferred: the xt pool has XT_BUFS slots, so an XPOSE
        # emitted too early would carry a WAR against reader chains that
        # appear later in program order (deadlock). ensure_staged() emits
        # each XPOSE on first demand / opportunistically once the readers
        # of the slot being evicted are all emitted.
        stagers = {}
        loads_emitted = []
        readers = {}
        for tname, b in [("q", 0), ("k", 0), ("v", 0), ("k", 1), ("v", 1),
                         ("q", 1), ("k", 2), ("v", 2), ("k", 3), ("v", 3),
                         ("q", 2), ("q", 3)]:
            ce = cast_est[block_cast_key(tname, b)]
            stagers[(tname, b)] = {
                "est": ce + 3584 + DMA_LINK + 1500, "done": False}

        def do_load(key):
            # XPOSE has a single semaphore-wait slot; reused pool slots would
            # need WAR+RAW, so a tiny DMA first touches the source chunk and
            # the destination tile, absorbing both waits.
            tname, b = key
            xb_d, _ = srcs[tname]
            lo = 512 * b
            xt = p_xt.tile([128, NKT, 512], dt.bfloat16, tag="xt", name="xt")
            if n_load_T[0] >= XT_BUFS:
                row = xb_d[lo : lo + 1, 0:NKT]
                nc.sync.dma_start(out=xt[:, :, 0:1], in_=row.to_broadcast([128, NKT]))
            n_load_T[0] += 1
            nc.sync.dma_start(out=xt[:], in_=xb_d[lo : lo + 512, :], transpose=True)
            xts[key] = xt

        def evict_target(key):
            if len(loads_emitted) < XT_BUFS:
                return None
            return loads_emitted[-XT_BUFS]

        def ensure_staged(key):
            st = stagers[key]
            if st["done"]:
                return
            tgt = evict_target(key)
            if tgt is not None:
                for cid in list(readers.get(tgt, [])):
                    run_chain(cid)
            st["done"] = True
            loads_emitted.append(key)
            do_load(key)

        # ================= QKV projection chains (PE fillers) ==============
        chains = {}
        chain_order = []

        def add_chain(cid, key, ready, dur, fn):
            chains[cid] = {"ready": ready, "dur": dur, "fn": fn, "done": False,
                           "key": key}
            chain_order.append(cid)
            readers.setdefault(key, []).append(cid)

        def emit_pqk(dst, wsb, tname, b, m):
            xt = xts[(tname, b)]
            ps = p_big.tile([128, 512], dt.float32, tag="psb", name="psb")
            for kt in range(NKT):
                nc.tensor.matmul(
                    ps[:], wsb[:, kt, 128 * m : 128 * (m + 1)], xt[:, kt, :],
                    start=(kt == 0), stop=(kt == NKT - 1),
                )
            nc.vector.tensor_copy(dst[m][:, 512 * b : 512 * (b + 1)], ps[:])

        def emit_pv(b, ti):
            xt = xts[("v", b)]
            tt = 4 * b + ti
            ps = p_big.tile([128, HGD], dt.float32, tag="psb", name="psb")
            for kt in range(NKT):
                nc.tensor.matmul(
                    ps[:], xt[:, kt, 128 * ti : 128 * (ti + 1)], wv_sb[:, kt, :],
                    start=(kt == 0), stop=(kt == NKT - 1),
                )
            nc.vector.tensor_copy(
                vh_ext[tt][:, :, 0:HD], ps.rearrange("p (h d) -> p h d", h=HPC)
            )

        for b in range(NQC):
            for m in range(NM):
                add_chain(("pk", b, m), ("k", b),
                          max(stagers[("k", b)]["est"], west["wk"]), 8 * 512 * MM,
                          lambda b=b, m=m: emit_pqk(khT, wk_sb, "k", b, m))
                add_chain(("pq", b, m), ("q", b),
                          max(stagers[("q", b)]["est"], west["wq"]), 8 * 512 * MM,
                          lambda b=b, m=m: emit_pqk(qhT, wq_sb, "q", b, m))
            for ti in range(4):
                add_chain(("pv", b, ti), ("v", b),
                          max(stagers[("v", b)]["est"], west["wv"]), 8 * 512 * MM,
                          lambda b=b, ti=ti: emit_pv(b, ti))

        # ================= attention unit emitters =========================
        windows = [(qc, pair) for qc in range(NQC) for pair in range(NM)]
        es_tiles = {}
        an_tiles = {}

        def emit_S(w, kt):
            qc, pair = windows[w]
            qsl = slice(512 * qc, 512 * (qc + 1))
            ksl = slice(128 * kt, 128 * (kt + 1))
            ps = p_ps.tile([128, 1024], dt.float32, tag="pss", name="pss")
            nc.tensor.matmul(ps[:, 0:512], khT[pair][0:64, ksl], qhT[pair][0:64, qsl],
                             start=True, stop=True)
            nc.tensor.matmul(ps[:, 512:1024], khT[pair][64:128, ksl], qhT[pair][64:128, qsl],
                             start=True, stop=True)
            es = p_exps.tile([128, 1024], dt.bfloat16, tag="es", name="es")
            nc.scalar.activation(es[:], ps[:], AF.Exp, scale=1.0 / 8.0)
            es_tiles[(w, kt)] = es

        def emit_drain(w, h, tqb, slot):
            # one closed accumulation: all 16 key tiles into one PSUM bank,
            # then normalize straight off the bank into the attn tile
            qc, pair = windows[w]
            av = av_banks[slot]
            for kt in range(NTT):
                nc.tensor.matmul(
                    av[:],
                    es_tiles[(w, kt)][:, 512 * h + 128 * tqb : 512 * h + 128 * (tqb + 1)],
                    vh_ext[kt][:, 2 * pair + h, :],
                    start=(kt == 0), stop=(kt == NTT - 1),
                )
            if qc not in an_tiles:
                an_tiles[qc] = p_an.tile(
                    [128, 4, NM, 2, HD], dt.bfloat16, tag="an", name="an")
            st = p_norm.tile([128, HD + 1], dt.float32, tag="st", name="st")
            nc.vector.tensor_copy(st[:], av[:])
            rc = p_norm.tile([128, 1], dt.float32, tag="rc", name="rc")
            nc.vector.reciprocal(rc[:], st[:, HD : HD + 1])
            nc.vector.tensor_scalar_mul(
                an_tiles[qc][:, tqb, pair, h, :], st[:, 0:HD], rc[:]
            )

        def emit_attn_dma(w):
            qc, pair = windows[w]
            an = an_tiles[qc]
            qsl = slice(512 * qc, 512 * (qc + 1))
            csl = slice(128 * pair, 128 * (pair + 1))
            if qc == NQC - 1:
                # last query chunk: per-window write+transpose keeps the tail
                # short (only window 15's own roundtrip after its norm)
                nc.sync.dma_start(
                    out=attn_d[qsl, csl].rearrange(
                        "(tb p) (h d) -> p tb h d", p=128, h=2),
                    in_=an[:, :, pair, :, :],
                )
                nc.sync.dma_start(
                    out=attnT[:, pair, qsl], in_=attn_d[qsl, csl], transpose=True)
                if pair == NM - 1:
                    an_tiles.pop(qc)
            elif pair == NM - 1:
                nc.sync.dma_start(
                    out=attn_d[qsl, :].rearrange(
                        "(tb p) (pr h d) -> p tb pr h d", p=128, pr=NM, h=2),
                    in_=an_tiles.pop(qc)[:],
                )
                nc.sync.dma_start(
                    out=attnT[:, :, qsl], in_=attn_d[qsl, :], transpose=True)

        def emit_proj(qc, tt):
            tsl = slice(128 * tt, 128 * (tt + 1))
            ysb = p_y.tile([128, EMB], dt.float32, tag="ysb", name="ysb")
            for ec in range(2):
                esl = slice(512 * ec, 512 * (ec + 1))
                ps = p_big.tile([128, 512], dt.float32, tag="psb", name="psb")
                for m in range(NM):
                    nc.tensor.matmul(ps[:], attnT[:, m, tsl], wp_sb[:, m, esl],
                                     start=(m == 0), stop=(m == NM - 1))
                nc.vector.tensor_copy(ysb[:, esl], ps[:])
            nc.gpsimd.dma_start(out=y[tsl, :], in_=ysb[:])

        # ================= build-time list scheduler =======================
        # Global S order honours staging epochs: epoch(w, kt) =
        # max(qc, kt//4); within an epoch, window-major so windows finish
        # (and release norm + projection work) as early as possible.
        # strict window-major: a window's 16 exp tiles must all be live when
        # its drains run, so windows complete one at a time (the scheduler
        # fills staging stalls on kt>=4 of window 0 with projection chains)
        s_order = [(w, kt) for w in range(NW) for kt in range(NTT)]

        def s_deps(w, kt):
            qc, pair = windows[w]
            return [("pq", qc, pair), ("pk", kt // 4, pair)]

        t_pe = [0.0]
        t_act = [0.0]
        t_dve = [0.0]
        av_free = [0.0]
        exp_end = {}

        plan_order = list(stagers.keys())

        def maybe_stage():
            # pre-issue the next planned XPOSE once the readers of the slot
            # it would evict have all been emitted
            for key in plan_order:
                if stagers[key]["done"]:
                    continue
                tgt = evict_target(key)
                if tgt is not None and any(
                    not chains[cid]["done"] for cid in readers.get(tgt, [])
                ):
                    return
                stagers[key]["done"] = True
                loads_emitted.append(key)
                do_load(key)
                return

        def urgent_chains():
            for key in plan_order:
                if not stagers[key]["done"]:
                    tgt = evict_target(key)
                    if tgt is None:
                        return []
                    return [c for c in readers.get(tgt, [])
                            if not chains[c]["done"]]
            return []

        def run_chain(cid):
            c = chains[cid]
            if c["done"]:
                return
            c["done"] = True
            ensure_staged(c["key"])
            t_pe[0] = max(t_pe[0], c["ready"]) + c["dur"]
            t_dve[0] = max(t_dve[0], t_pe[0] + SEM) + DVE_COPY
            c["fn"]()

        def s_ready_est(idx, w, kt):
            r = 0.0
            for cid in s_deps(w, kt):
                c = chains[cid]
                r = max(r, (c["ready"] + c["dur"] if not c["done"] else 0.0))
            if idx >= 2:
                r = max(r, exp_end[idx - 2] + SEM)
            return r

        proj_q = []          # [ready est, qc, tt]
        drain_q = []         # (w, h, tqb) fifo
        s_run = [0]
        s_idx = [0]
        d_idx = [0]
        drained_windows = [0]
        evac_est = [0.0, 0.0, 0.0]
        win_exp_end = {}
        # position of each window's last S unit in s_order
        last_s_pos = {}
        for i, (w, kt) in enumerate(s_order):
            last_s_pos[w] = max(last_s_pos.get(w, -1), i)

        def do_S():
            w, kt = s_order[s_idx[0]]
            for cid in s_deps(w, kt):
                run_chain(cid)
            t_pe[0] = max(t_pe[0], s_ready_est(s_idx[0], w, kt)) + 2 * 512 * MM
            emit_S(w, kt)
            t_act[0] = max(t_act[0], t_pe[0] + SEM) + EXP_NS
            exp_end[s_idx[0]] = t_act[0]
            if s_idx[0] == last_s_pos[w]:
                win_exp_end[w] = t_act[0]
                for tqb in range(4):
                    for h in range(2):
                        drain_q.append((w, h, tqb))
            s_idx[0] += 1
            s_run[0] += 1

        def drain_ready_est():
            w, h, tqb = drain_q[0]
            slot = d_idx[0] % 3
            r = max(win_exp_end[w] + SEM, evac_est[slot])
            for ti in range(4):
                for b in range(NQC):
                    c = chains[("pv", b, ti)]
                    if not c["done"]:
                        r = max(r, c["ready"] + c["dur"])
            return r

        def do_drain():
            w, h, tqb = drain_q.pop(0)
            qc, pair = windows[w]
            slot = d_idx[0] % 3
            for b in range(NQC):
                for ti in range(4):
                    run_chain(("pv", b, ti))
            t_pe[0] = max(t_pe[0], win_exp_end[w] + SEM, evac_est[slot]) + 16 * 65 * MM
            s_run[0] = 0
            emit_drain(w, h, tqb, slot)
            t_dve[0] = max(t_dve[0], t_pe[0] + SEM) + 1100
            evac_est[slot] = t_dve[0] - 700 + SEM
            d_idx[0] += 1
            if h == 1 and tqb == 3:
                for kt in range(NTT):
                    es_tiles.pop((w, kt))
                drained_windows[0] += 1
                emit_attn_dma(w)
                if pair == NM - 1:
                    rdy = t_dve[0] + 2 * DMA_LINK + 6000
                    for tt in range(4 * qc, 4 * qc + 4):
                        proj_q.append([rdy, qc, tt])

        PROJ_RESERVE = 4

        def next_filler():
            for cid in urgent_chains():
                if chains[cid]["ready"] <= t_pe[0] + 400:
                    return ("chain", cid)
            for cid in chain_order:
                c = chains[cid]
                if not c["done"] and c["ready"] <= t_pe[0] + 100:
                    return ("chain", cid)
            # hold qc2's tiles as tail filler (they cover the wait for the
            # last window's transpose); everything else serves freely
            for it in proj_q:
                if it[0] <= t_pe[0] + 100 and (it[1] != 2 or drained_windows[0] >= NW - 1):
                    return ("proj", it)
            return None

        def do_filler(f):
            kind, it = f
            if kind == "chain":
                run_chain(it)
            else:
                proj_q.remove(it)
                t_pe[0] = max(t_pe[0], it[0]) + 8 * 512 * MM
                emit_proj(it[1], it[2])
                t_dve[0] = max(t_dve[0], t_pe[0] + SEM) + 2 * DVE_COPY

        NSU = len(s_order)
        while s_idx[0] < NSU or drain_q or proj_q or not all(
            c["done"] for c in chains.values()
        ):
            maybe_stage()
            es_ok = (s_idx[0] - 16 * drained_windows[0]) < ES_BUFS - 1
            can_S = s_idx[0] < NSU and es_ok
            can_D = bool(drain_q)
            s_stall = (
                max(0.0, s_ready_est(s_idx[0], *s_order[s_idx[0]]) - t_pe[0])
                if can_S else float("inf")
            )
            d_stall = (
                max(0.0, drain_ready_est() - t_pe[0]) if can_D else float("inf")
            )
            if s_idx[0] < NSU and not es_ok and can_D:
                do_drain()
                continue
            if can_D and d_stall <= 30 and (s_run[0] >= 2 or s_stall > 30):
                do_drain()
                continue
            if can_S and s_stall <= 30:
                do_S()
                continue
            if can_D and d_stall <= 30:
                do_drain()
                continue
            f = next_filler()
            if f is not None:
                do_filler(f)
                continue
            if can_D and d_stall <= s_stall:
                do_drain()
            elif can_S:
                do_S()
            elif can_D:
                do_drain()
            elif proj_q:
                it = min(proj_q, key=lambda x: x[0])
                do_filler(("proj", it))
            else:
                rem = [cid for cid in chain_order if not chains[cid]["done"]]
                run_chain(min(rem, key=lambda c: chains[c]["ready"]))

    nc.compile()
    return nc


def _get_nc():
    if "nc" not in _CACHE:
        _CACHE["nc"] = _build_nc()
    return _CACHE["nc"]


def core_input_map(k, q, v, w_key, w_query, w_value, w_proj, core):
    b, g = core // 2, core % 2
    sl = slice(g * HGD, (g + 1) * HGD)
    f32 = np.float32
    return {
        "xq": np.ascontiguousarray(q[b], dtype=f32),
        "xk": np.ascontiguousarray(k[b], dtype=f32),
        "xv": np.ascontiguousarray(v[b], dtype=f32),
        "wq": np.ascontiguousarray(w_query[:, sl], dtype=f32),
        "wk": np.ascontiguousarray(w_key[:, sl], dtype=f32),
        "wv": np.ascontiguousarray(w_value[:, sl], dtype=f32),
        "wp": np.ascontiguousarray(w_proj[sl, :], dtype=f32),
    }


def kernel(k, q, v, w_key, w_query, w_value, w_proj):
    from concourse.bass_utils import run_bass_kernel_spmd

    nc = _get_nc()
    in_maps = [
        core_input_map(k, q, v, w_key, w_query, w_value, w_proj, c) for c in range(8)
    ]
    res = run_bass_kernel_spmd(nc, in_maps, list(range(8))).results
    out = np.empty((B, T, EMB), np.float32)
    for b in range(B):
        out[b] = res[2 * b]["y"] + res[2 * b + 1]["y"]
    return out


# revision 35
# speedup vs baseline: 1.0856x; 1.0856x over previous
"""Multi-head attention kernel for Trainium2, 8 NeuronCores.

Problem: B=4, T=2048, D_in=1024, 16 heads x 64 dim, E=1024 (fp32).

Sharding: (batch x head-group). Core c handles batch b=c//2 and head-group
g=c%2 (8 heads, 512 qk/v dims). Each core computes its batch's QKV
projections restricted to its heads, full attention for those heads, and a
partial output projection. The host sums the two partial projections per
batch (the only cross-core reduction) and stacks batches.

Per-core dataflow (all matmuls bf16 inputs, fp32 PSUM accumulation):
  xT      = dma-xbar-transpose(cast_bf16(x))            [1024, 2048] per tensor
  qhT/khT = w.T @ xT   (weights stationary)             [512, 2048]
  vh      = xT.T @ wv  (x stationary) + ones column     [2048, 8*65]
  S^T     = khT_h.T @ qhT_h per head pair               PSUM [128,1024]
  expS    = ACT exp(S^T/8) -> bf16 SBUF                 (the softmax exp)
  AV      = vh_ext.T @ expS  (accumulate over k tiles)  PSUM [65, 512]
            row 64 = softmax denominator (ones column)
  attnT   = (AV[0:64] * 1/denom) -> bf16                [64, 2048] per head
  y      += attnT_h.T @ wp_h  (K=64 contraction)        [2048, 1024] fp32
"""

import sys

import numpy as np

if "/opt/trn_rl_repo" not in sys.path:
    sys.path.insert(0, "/opt/trn_rl_repo")

B, T, DIN = 4, 2048, 1024
NH, HD, EMB = 16, 64, 1024
HGD = 512          # per-core qk/v dims (8 heads * 64)
NKT = DIN // 128   # 8  input-dim k tiles
NQC = T // 512     # 4  t chunks of 512
NTT = T // 128     # 16 t tiles of 128
NM = HGD // 128    # 4  head-pair m tiles
HPC = 8            # heads per core

_CACHE = {}

# build-time tunables (model-guided sweeps)
TUNE = {"CAP": 12, "EXPS_BUFS": 18, "DRAIN": 16}


def _build_nc():
    import concourse.bacc as bacc
    import concourse.bass as bass
    import concourse.mybir as mybir
    import concourse.tile as tile

    dt = mybir.dt
    AF = mybir.ActivationFunctionType

    nc = bacc.Bacc("TRN2", target_bir_lowering=False, debug=False)
    xq = nc.declare_dram_parameter("xq", [T, DIN], dt.float32, isOutput=False)
    xk = nc.declare_dram_parameter("xk", [T, DIN], dt.float32, isOutput=False)
    xv = nc.declare_dram_parameter("xv", [T, DIN], dt.float32, isOutput=False)
    wq = nc.declare_dram_parameter("wq", [DIN, HGD], dt.float32, isOutput=False)
    wk = nc.declare_dram_parameter("wk", [DIN, HGD], dt.float32, isOutput=False)
    wv = nc.declare_dram_parameter("wv", [DIN, HGD], dt.float32, isOutput=False)
    wp = nc.declare_dram_parameter("wp", [HGD, EMB], dt.float32, isOutput=False)
    y = nc.declare_dram_parameter("y", [T, EMB], dt.float32, isOutput=True)

    with tile.TileContext(nc) as tc:
        from contextlib import ExitStack

        with ExitStack() as ctx:
            p_w = ctx.enter_context(tc.tile_pool(name="weights", bufs=1))
            p_xt = ctx.enter_context(tc.tile_pool(name="xt", bufs=4))
            p_qkh = ctx.enter_context(tc.tile_pool(name="qkh", bufs=1))
            p_vh = ctx.enter_context(tc.tile_pool(name="vh", bufs=1))
            p_exps = ctx.enter_context(tc.tile_pool(name="exps", bufs=TUNE["EXPS_BUFS"]))
            p_attn = ctx.enter_context(tc.tile_pool(name="attn", bufs=1))
            p_norm = ctx.enter_context(tc.tile_pool(name="norm", bufs=4))
            p_y = ctx.enter_context(tc.tile_pool(name="ysb", bufs=2))
            p_ps = ctx.enter_context(tc.tile_pool(name="psum_s", bufs=2, space="PSUM"))
            p_av = ctx.enter_context(tc.tile_pool(name="psum_av", bufs=1, space="PSUM"))
            p_big = ctx.enter_context(tc.tile_pool(name="psum_big", bufs=2, space="PSUM"))

            # DRAM scratch used to partition-broadcast softmax denominators
            nscr = nc.dram_tensor("nscr", [32, 512], dt.float32)
            # bf16 copies of the inputs (DRAM->DRAM cast), transposed-read later
            xqb = nc.dram_tensor("xqb", [T, DIN], dt.bfloat16)
            xkb = nc.dram_tensor("xkb", [T, DIN], dt.bfloat16)
            xvb = nc.dram_tensor("xvb", [T, DIN], dt.bfloat16)

            # --- weights: cast to bf16 during SWDGE DMA, k-tiled layouts ---
            # w*_sb[p, kt, n] = w[kt*128 + p, n]
            wq_sb = p_w.tile([128, NKT, HGD], dt.bfloat16, tag="wq")
            wk_sb = p_w.tile([128, NKT, HGD], dt.bfloat16, tag="wk")
            wv_sb = p_w.tile([128, NKT, HGD], dt.bfloat16, tag="wv")
            # wp pair-tiled to match attnT: wp_sb[p, m, e] = wp[m*128+p, e]
            wp_sb = p_w.tile([128, NM, EMB], dt.bfloat16, tag="wp")

            # persistent activations
            qhT = [p_qkh.tile([128, T], dt.bfloat16, tag=f"qhT{m}", name=f"qhT{m}") for m in range(NM)]
            khT = [p_qkh.tile([128, T], dt.bfloat16, tag=f"khT{m}", name=f"khT{m}") for m in range(NM)]
            # vh_ext[t, h, 0:64] = vh, vh_ext[t, h, 64] = 1.0 (softmax denom)
            vh_ext = [p_vh.tile([128, HPC, HD + 1], dt.bfloat16, tag=f"vh{tt}", name=f"vh{tt}") for tt in range(NTT)]
            for tt in range(NTT):
                nc.vector.memset(vh_ext[tt][:, :, HD : HD + 1], 1.0)
            # attnT[m]: head 2m in rows 0:64, head 2m+1 in rows 64:128
            attnT = [p_attn.tile([128, T], dt.bfloat16, tag=f"at{m}", name=f"at{m}") for m in range(NM)]

            # --- phase 1: loads, transposes, projections (per 512-token block) ---
            # cast f32 -> bf16 into DRAM scratch (SWDGE), chunked for overlap.
            # First the block-0 casts + the weights they unblock, so the first
            # projection matmuls start as early as possible.
            tsl0 = slice(0, 512)
            nc.gpsimd.dma_start(out=xkb[tsl0, :], in_=xk[tsl0, :])
            wk_r = wk.rearrange("(kt p) n -> p kt n", p=128)
            nc.gpsimd.dma_start(out=wk_sb[:, :, 0:128], in_=wk_r[:, :, 0:128])
            nc.gpsimd.dma_start(out=wk_sb[:, :, 128:HGD], in_=wk_r[:, :, 128:HGD])
            nc.gpsimd.dma_start(out=xvb[tsl0, :], in_=xv[tsl0, :])
            nc.gpsimd.dma_start(out=wv_sb[:], in_=wv.rearrange("(kt p) n -> p kt n", p=128))
            nc.gpsimd.dma_start(out=xqb[tsl0, :], in_=xq[tsl0, :])
            nc.gpsimd.dma_start(out=wq_sb[:], in_=wq.rearrange("(kt p) n -> p kt n", p=128))
            for qcb in range(1, NQC):
                tsl = slice(512 * qcb, 512 * (qcb + 1))
                nc.gpsimd.dma_start(out=xkb[tsl, :], in_=xk[tsl, :])
                nc.gpsimd.dma_start(out=xvb[tsl, :], in_=xv[tsl, :])
                nc.gpsimd.dma_start(out=xqb[tsl, :], in_=xq[tsl, :])
            nc.gpsimd.dma_start(out=wp_sb[:], in_=wp.rearrange("(m p) e -> p m e", p=128))

            n_load_T = [0]

            def load_T(xb, qcb):
                """xbar-transpose one 512-token block from bf16 DRAM.

                xt[p, kt, t] = x[512*qcb + t, kt*128 + p]

                The XPOSE ISA instruction has a single semaphore-wait slot.
                Fresh pool slots only wait on the source cast (1 wait, fine);
                reused slots would also carry a WAR wait, so for those a tiny
                ordinary DMA first touches the source chunk and the whole
                destination tile, absorbing both waits.
                """
                xt = p_xt.tile([128, NKT, 512], dt.bfloat16, tag="xt")
                if n_load_T[0] >= 4:  # p_xt bufs exhausted -> slot reuse
                    row = xb[512 * qcb : 512 * qcb + 1, 0:NKT]
                    nc.sync.dma_start(
                        out=xt[:, :, 0:1], in_=row.to_broadcast([128, NKT])
                    )
                n_load_T[0] += 1
                nc.sync.dma_start(
                    out=xt[:], in_=xb[512 * qcb : 512 * (qcb + 1), :], transpose=True
                )
                return xt

            # ---- attention emission state (interleaved with phase 1) ----
            # Window = (qc, pair): 2 heads x 512 queries, accumulated over 16
            # key tiles. Only one window owns the AV PSUM accumulators at a
            # time; later windows run S+exp ahead into SBUF slots (lookahead
            # bounded by the exps pool) so the scalar engine never idles.
            windows = [(qc, pair) for qc in range(NQC) for pair in range(NM)]
            sdone = {w: 0 for w in windows}
            buf = {w: [] for w in windows}
            av_tiles = {}
            state = {"open": 0, "inflight": 0}
            CAP = TUNE["CAP"]

            def emit_s_exp(w):
                qc, pair = w
                kt = sdone[w]
                qsl_w = slice(512 * qc, 512 * (qc + 1))
                ksl = slice(128 * kt, 128 * (kt + 1))
                ps = p_ps.tile([128, 1024], dt.float32, tag="pss", name="pss")
                nc.tensor.matmul(
                    ps[:, 0:512], khT[pair][0:64, ksl], qhT[pair][0:64, qsl_w],
                    start=True, stop=True,
                )
                nc.tensor.matmul(
                    ps[:, 512:1024], khT[pair][64:128, ksl], qhT[pair][64:128, qsl_w],
                    start=True, stop=True,
                )
                es = p_exps.tile([128, 1024], dt.bfloat16, tag="es", name="es")
                nc.scalar.activation(es[:], ps[:], AF.Exp, scale=1.0 / 8.0)
                buf[w].append((kt, es))
                sdone[w] += 1
                state["inflight"] += 1

            def emit_av_drain(w):
                qc, pair = w
                if w not in av_tiles:
                    av_a = p_av.tile([HD + 1, 512], dt.float32, tag="ava", name="ava")
                    av_b = p_av.tile([HD + 1, 512], dt.float32, tag="avb", name="avb")
                    av_tiles[w] = (av_a, av_b)
                av_a, av_b = av_tiles[w]
                for kt, es in buf[w]:
                    nc.tensor.matmul(
                        av_a[:], vh_ext[kt][:, 2 * pair, :], es[:, 0:512],
                        start=(kt == 0), stop=(kt == NTT - 1),
                    )
                    nc.tensor.matmul(
                        av_b[:], vh_ext[kt][:, 2 * pair + 1, :], es[:, 512:1024],
                        start=(kt == 0), stop=(kt == NTT - 1),
                    )
                    state["inflight"] -= 1
                buf[w].clear()

            def emit_norm(w):
                qc, pair = w
                qsl_w = slice(512 * qc, 512 * (qc + 1))
                av_a, av_b = av_tiles.pop(w)
                for h2, av in ((0, av_a), (1, av_b)):
                    i = (pair * NQC + qc) * 2 + h2
                    # evacuate the accumulator to SBUF so the PSUM bank frees
                    # immediately; normalize off the staged copy
                    st = p_norm.tile([HD + 1, 512], dt.float32, tag=f"st{h2}", name="st")
                    nc.vector.tensor_copy(st[:], av[:])
                    rc = p_norm.tile([1, 512], dt.float32, tag="rc", name="rc")
                    nc.vector.reciprocal(rc[:], st[HD : HD + 1, :])
                    nc.sync.dma_start(out=nscr[i : i + 1, :], in_=rc[:])
                    rb = p_norm.tile([64, 512], dt.float32, tag="rb", name="rb")
                    nc.sync.dma_start(
                        out=rb[:], in_=nscr[i : i + 1, :].to_broadcast([64, 512])
                    )
                    nc.vector.tensor_mul(
                        attnT[pair][64 * h2 : 64 * h2 + 64, qsl_w], st[0:HD, :], rb[:]
                    )

            def emit_proj_qc(qc):
                for tt in range(4 * qc, 4 * qc + 4):
                    tsl = slice(128 * tt, 128 * (tt + 1))
                    ysb = p_y.tile([128, EMB], dt.float32, tag="ysb", name="ysb")
                    for ec in range(2):
                        esl = slice(512 * ec, 512 * (ec + 1))
                        ps = p_big.tile([128, 512], dt.float32, tag="psb", name="psb")
                        for m in range(NM):
                            nc.tensor.matmul(
                                ps[:],
                                attnT[m][:, tsl],
                                wp_sb[:, m, esl],
                                start=(m == 0),
                                stop=(m == NM - 1),
                            )
                        nc.vector.tensor_copy(ysb[:, esl], ps[:])
                    nc.sync.dma_start(out=y[tsl, :], in_=ysb[:])

            def emit_attn_progress(hi):
                # advance the open window as far as data allows
                while state["open"] < len(windows):
                    w = windows[state["open"]]
                    while sdone[w] < hi:
                        emit_s_exp(w)
                        if len(buf[w]) >= TUNE["DRAIN"]:
                            emit_av_drain(w)
                    emit_av_drain(w)
                    if sdone[w] == NTT:
                        emit_norm(w)
                        qc, pair = w
                        state["open"] += 1
                        if pair == NM - 1:
                            emit_proj_qc(qc)
                    else:
                        break
                # lookahead S+exp into free slots
                li = state["open"] + 1
                while state["inflight"] < CAP and li < len(windows):
                    w2 = windows[li]
                    if sdone[w2] < hi:
                        emit_s_exp(w2)
                    else:
                        li += 1

            for qcb in range(NQC):
                xkT = load_T(xkb, qcb)
                xvT = load_T(xvb, qcb)
                xqT = load_T(xqb, qcb)
                qsl = slice(512 * qcb, 512 * (qcb + 1))

                def pk(m):
                    ps = p_big.tile([128, 512], dt.float32, tag="psb", name="psb")
                    for kt in range(NKT):
                        nc.tensor.matmul(
                            ps[:],
                            wk_sb[:, kt, 128 * m : 128 * (m + 1)],
                            xkT[:, kt, :],
                            start=(kt == 0),
                            stop=(kt == NKT - 1),
                        )
                    nc.vector.tensor_copy(khT[m][:, qsl], ps[:])

                def pv(ti):
                    tt = 4 * qcb + ti
                    ps = p_big.tile([128, 512], dt.float32, tag="psb", name="psb")
                    for kt in range(NKT):
                        nc.tensor.matmul(
                            ps[:],
                            xvT[:, kt, 128 * ti : 128 * (ti + 1)],
                            wv_sb[:, kt, :],
                            start=(kt == 0),
                            stop=(kt == NKT - 1),
                        )
                    nc.vector.tensor_copy(
                        vh_ext[tt][:, :, 0:HD],
                        ps.rearrange("p (h d) -> p h d", h=HPC),
                    )

                def pq(m):
                    ps = p_big.tile([128, 512], dt.float32, tag="psb", name="psb")
                    for kt in range(NKT):
                        nc.tensor.matmul(
                            ps[:],
                            wq_sb[:, kt, 128 * m : 128 * (m + 1)],
                            xqT[:, kt, :],
                            start=(kt == 0),
                            stop=(kt == NKT - 1),
                        )
                    nc.vector.tensor_copy(qhT[m][:, qsl], ps[:])

                for i in range(4):
                    pk(i)
                for i in range(4):
                    pv(i)
                for i in range(4):
                    pq(i)
                emit_attn_progress(4 * (qcb + 1))

    nc.compile()
    return nc


def _get_nc():
    if "nc" not in _CACHE:
        _CACHE["nc"] = _build_nc()
    return _CACHE["nc"]


def core_input_map(k, q, v, w_key, w_query, w_value, w_proj, core):
    b, g = core // 2, core % 2
    sl = slice(g * HGD, (g + 1) * HGD)
    f32 = np.float32
    return {
        "xq": np.ascontiguousarray(q[b], dtype=f32),
        "xk": np.ascontiguousarray(k[b], dtype=f32),
        "xv": np.ascontiguousarray(v[b], dtype=f32),
        "wq": np.ascontiguousarray(w_query[:, sl], dtype=f32),
        "wk": np.ascontiguousarray(w_key[:, sl], dtype=f32),
        "wv": np.ascontiguousarray(w_value[:, sl], dtype=f32),
        "wp": np.ascontiguousarray(w_proj[sl, :], dtype=f32),
    }


def kernel(k, q, v, w_key, w_query, w_value, w_proj):
    from concourse.bass_utils import run_bass_kernel_spmd

    nc = _get_nc()
    in_maps = [
        core_input_map(k, q, v, w_key, w_query, w_value, w_proj, c) for c in range(8)
    ]
    res = run_bass_kernel_spmd(nc, in_maps, list(range(8))).results
    out = np.empty((B, T, EMB), np.float32)
    for b in range(B):
        out[b] = res[2 * b]["y"] + res[2 * b + 1]["y"]
    return out

